# revision 23
# baseline (speedup 1.0000x reference)
"""Trainium2 Bass kernel for nn_ComparisonLayer.

Math (reference):
    x: [L=512, B=2, D=256] -> transpose to [B, L, D], layernorm over D
    a = xn @ w1.T + b1                  # [B, L, C=128]
    b = xn @ w2.T + b2                  # [B, L, C]
    out[b,i,j,o] = sum_c a[b,i,c]*b[b,j,c]*w3[o,c] + b3[o]
                 + sum_c (a[b,i,c]-b[b,j,c])*w4[o,c]      # [B, L, L, O=64]

Host-side input folding (exact):
  - norm_w/norm_b fold into the GEMM weights/biases: w1<-w1*norm_w,
    b1<-b1+w1@norm_b (same for w2/b2), so the device only standardizes x.
  - b3 folds into a second b-bias: host computes the exact min-norm t with
    w4 @ t = b3 (w4 full row rank; b3 is zeros in the reference anyway) and
    passes b2s = b2' - t. Then -(bTs @ w4.T) = b3 - (b @ w4.T).

Per batch, per core j-slice of 64 columns, each [128 i, 512 (j,o)] PSUM tile:
    MM_B: lhsT=ones[128,128],  rhs=V4m[c,(j,o)] = bTs[c,j] * (-w4T[c,o])
          -> b3[o] - (b @ w4.T)[j,o]
    (MM_C: lhsT=aT i-tile,     rhs=w4T j-broadcast, only for ACT-copied tiles
          -> (a @ w4.T)[i,o]; DVE-copied tiles get this term as a fused
          broadcast add of A4 during the PSUM->SBUF epilogue)
    MM_A: lhsT=aT i-tile,      rhs=V3[c,(j,o)] = bT[c,j] * w3T[c,o]
          -> sum_c a[i,c] b[j,c] w3[o,c]
All matmuls run in float32r (1 cycle/row at N=512 vs 4 for fp32).

Sharding: second L (the j axis) split across the 8 cores; each core gets the
full x (for a) plus its own 64-row slice xj (for b) and returns
out[:, :, 64k:64k+64, :]; the host concatenates along axis 2.
"""

import sys

if "/opt/trn_rl_repo" not in sys.path:
    sys.path.insert(0, "/opt/trn_rl_repo")

from contextlib import ExitStack

import numpy as np

import concourse.bacc as bacc
import concourse.mybir as mybir
import concourse.tile as tile
from concourse.alu_op_type import AluOpType
from concourse.bass_utils import run_bass_kernel_spmd
from concourse.masks import make_identity

L, B, D = 512, 2, 256
C, O = 128, 64
NCORES = 8
JS = L // NCORES  # 64 j's per core
JB = 8  # j's per 512-wide chunk
F32 = mybir.dt.float32
F32R = mybir.dt.float32r
ACT_COPY = mybir.ActivationFunctionType.Copy
ACT_IDENT = mybir.ActivationFunctionType.Identity


def build_nc():
    nc = bacc.Bacc("TRN2", target_bir_lowering=False)

    x = nc.dram_tensor("x", [L, B, D], F32, kind="ExternalInput")
    xj = nc.dram_tensor("xj", [JS, B, D], F32, kind="ExternalInput")
    w12 = nc.dram_tensor("w12", [2, C, D], F32, kind="ExternalInput")
    w34 = nc.dram_tensor("w34", [2, O, C], F32, kind="ExternalInput")
    b123 = nc.dram_tensor("b123", [C, 3], F32, kind="ExternalInput")
    out = nc.dram_tensor("out", [B, L, JS, O], F32, kind="ExternalOutput")

    with tile.TileContext(nc) as tc, ExitStack() as ctx:
        consts = ctx.enter_context(tc.tile_pool(name="consts", bufs=1))
        big = ctx.enter_context(tc.tile_pool(name="big", bufs=1))
        xwork = ctx.enter_context(tc.tile_pool(name="xwork", bufs=7))
        stats = ctx.enter_context(tc.tile_pool(name="stats", bufs=8))
        ps_pool = ctx.enter_context(tc.tile_pool(name="ps", bufs=3, space="PSUM"))
        ps_small = ctx.enter_context(tc.tile_pool(name="pss", bufs=2, space="PSUM"))
        stage_pool = ctx.enter_context(tc.tile_pool(name="stage", bufs=3))

        # ---- local constants ----
        warm = consts.tile([1, 1], F32)
        nc.vector.memset(warm, 1.0)
        nc.scalar.activation(out=warm, in_=warm,
                             func=mybir.ActivationFunctionType.Sqrt)
        nc.scalar.activation(out=warm, in_=warm, func=ACT_IDENT)
        nc.scalar.activation(out=warm, in_=warm, func=ACT_COPY)
        ident = consts.tile([128, 128], F32)
        make_identity(nc, ident)
        ones_f32 = consts.tile([128, 128], F32)
        nc.vector.memset(ones_f32, 1.0)
        ones128 = consts.tile([128, 128], F32R)
        nc.vector.tensor_copy(out=ones128, in_=ones_f32)
        eps_tile = consts.tile([128, 1], F32)
        nc.vector.memset(eps_tile, 1e-5)

        # ---- input loads: 5 coalesced DMAs (HWDGE setup is ~0.6us each) ----
        xjall = xwork.tile([JS, B, D], F32, name="xjall", bufs=1)
        nc.sync.dma_start(out=xjall, in_=xj.ap())
        w12_sb = consts.tile([C, 2, D], F32)
        nc.scalar.dma_start(out=w12_sb, in_=w12.ap().transpose([1, 0, 2]))
        w1_sb = w12_sb[:, 0, :]
        w2_sb = w12_sb[:, 1, :]
        w34_sb = consts.tile([O, 2, C], F32)
        nc.scalar.dma_start(out=w34_sb, in_=w34.ap().transpose([1, 0, 2]))
        w3_sb = w34_sb[:, 0, :]
        w4_sb = w34_sb[:, 1, :]
        b123_sb = consts.tile([C, 3], F32)
        nc.scalar.dma_start(out=b123_sb, in_=b123.ap())
        b1c = b123_sb[:, 0:1]
        b2c = b123_sb[:, 1:2]
        b2sc = b123_sb[:, 2:3]
        xj_t = [xjall[:, bb, :] for bb in range(B)]
        xall = [xwork.tile([128, 4, D], F32, name=f"xall{b_}", bufs=1)
                for b_ in range(B)]
        for bb in range(B):
            nc.sync.dma_start(
                out=xall[bb],
                in_=x.ap().rearrange("(lt p) b d -> p lt b d", p=128)[:, :, bb, :])
        x_t = {(bb, lt): xall[bb][:, lt, :] for bb in range(B) for lt in range(4)}

        # ---- weight transposes (PE stream head) ----
        def pe_transpose(dst_sb, src_ap, rows, cols):
            # dst[cols, rows] = src[rows, cols].T ; rows<=128, cols<=128
            pst = ps_small.tile([128, 128], F32, tag="ps_sm")
            nc.tensor.transpose(out=pst[:cols, :rows], in_=src_ap,
                                identity=ident[:rows, :rows])
            nc.scalar.activation(out=dst_sb, in_=pst[:cols, :rows], func=ACT_COPY)

        w2sT = [consts.tile([128, C], F32R, name=f"w2sT{i}") for i in range(2)]
        w1sT = [consts.tile([128, C], F32R, name=f"w1sT{i}") for i in range(2)]
        for dt_ in range(2):
            pe_transpose(w2sT[dt_], w2_sb[:, dt_ * 128:(dt_ + 1) * 128], C, 128)
            pe_transpose(w1sT[dt_], w1_sb[:, dt_ * 128:(dt_ + 1) * 128], C, 128)
        w3T = consts.tile([C, O], F32)
        pe_transpose(w3T, w3_sb, O, C)
        w4T = consts.tile([C, O], F32)
        pe_transpose(w4T, w4_sb, O, C)
        w4Tn = consts.tile([C, O], F32)  # -w4T, for MM_B via bTs
        nc.vector.tensor_scalar(out=w4Tn, in0=w4T, scalar1=-1.0, scalar2=None,
                                op0=AluOpType.mult)
        w4Tr = consts.tile([C, O], F32R)  # f32r copy, rhs of MM_C
        nc.scalar.activation(out=w4Tr, in_=w4T, func=ACT_COPY)

        # ---- layernorm pieces ----
        def ln_normalize(xt, nrows, norm_eng):
            """bn stats + (x - mu) * rstd; normalize on ACT (0) or Pool (1)."""
            mv = stats.tile([nrows, nc.vector.BN_AGGR_DIM], F32, tag="mv")
            st = stats.tile([nrows, nc.vector.BN_STATS_DIM], F32, tag="st")
            nc.vector.bn_stats(out=st, in_=xt)
            nc.vector.bn_aggr(out=mv, in_=st)
            rstd = stats.tile([nrows, 1], F32, tag="rstd")
            nc.scalar.activation(
                out=rstd, in_=mv[:, 1:2], func=mybir.ActivationFunctionType.Sqrt,
                bias=eps_tile[:nrows], scale=1.0)
            nc.vector.reciprocal(out=rstd, in_=rstd)
            nmr = stats.tile([nrows, 1], F32, tag="nmr")  # -mu * rstd
            nc.vector.tensor_tensor(out=nmr, in0=mv[:, 0:1], in1=rstd,
                                    op=AluOpType.mult)
            nc.vector.tensor_scalar(out=nmr, in0=nmr, scalar1=-1.0, scalar2=None,
                                    op0=AluOpType.mult)
            xn = xwork.tile([nrows, D], F32, tag="xn")
            if norm_eng == 0:
                nc.scalar.activation(out=xn, in_=xt, func=ACT_IDENT, bias=nmr,
                                     scale=rstd)
            elif norm_eng == 1:
                nc.gpsimd.tensor_scalar(out=xn, in0=xt, scalar1=rstd, scalar2=nmr,
                                        op0=AluOpType.mult, op1=AluOpType.add)
            else:
                nc.vector.tensor_scalar(out=xn, in0=xt, scalar1=rstd, scalar2=nmr,
                                        op0=AluOpType.mult, op1=AluOpType.add)
            return xn

        def transpose_pair(xn, nrows, dstT, col0):
            """Transpose xn [nrows, 256] into dstT [128, (dt 2, l)] columns
            col0:col0+nrows with one fused PSUM->SBUF copy (ACT)."""
            pst = ps_small.tile([128, 256], F32, tag="ps_sm")
            for dt_ in range(2):
                nc.tensor.transpose(
                    out=pst[:, dt_ * 128:dt_ * 128 + nrows],
                    in_=xn[:, dt_ * 128:(dt_ + 1) * 128],
                    identity=ident[:nrows, :nrows])
            dst = dstT.rearrange("p (t l) -> p t l", t=2)[:, :, col0:col0 + nrows]
            src = pst.rearrange("p (t l) -> p t l", t=2)[:, :, :nrows]
            nc.scalar.activation(out=dst, in_=src, func=ACT_COPY)

        # ---- xj -> xjT -> bT / bTs ----
        xjT = [big.tile([128, 2 * JS], F32R, name=f"xjT{b_}") for b_ in range(B)]
        bT = [big.tile([C, JS], F32, name=f"bT{b_}") for b_ in range(B)]
        bTs = [big.tile([C, JS], F32, name=f"bTs{b_}") for b_ in range(B)]
        for bb in range(B):
            xn = ln_normalize(xj_t[bb], JS, 0 if bb == 0 else 2)
            transpose_pair(xn, JS, xjT[bb], 0)
            psb = ps_small.tile([C, JS], F32, tag="ps_sm")
            xjT3 = xjT[bb].rearrange("p (t l) -> p t l", t=2)
            for dt_ in range(2):
                nc.tensor.matmul(out=psb, lhsT=w2sT[dt_], rhs=xjT3[:, dt_, :],
                                 start=(dt_ == 0), stop=(dt_ == 1))
            nc.scalar.activation(out=bT[bb], in_=psb, func=ACT_IDENT, bias=b2c)
            nc.scalar.activation(out=bTs[bb], in_=psb, func=ACT_IDENT, bias=b2sc)

        # ---- x layernorm + transpose + per-slice aT / A4 ----
        # b0 tiles normalize on ACT, b1 tiles on Pool, so Pool reaches the V4m
        # chunks early while ACT drives b0's critical path.
        xnT = [big.tile([128, 2 * L], F32R, name=f"xnT{b_}") for b_ in range(B)]
        aT = [big.tile([C, L], F32R, name=f"aT{b_}") for b_ in range(B)]
        A4 = [[big.tile([128, O], F32, name=f"A4_{b_}_{i}") for i in range(4)]
              for b_ in range(B)]
        def emit_x_pe_side(bb, lt, xn):
            transpose_pair(xn, 128, xnT[bb], lt * 128)
            # aT slice for this (bb, lt): columns lt*128:(lt+1)*128
            psa = ps_small.tile([C, 128], F32, tag="ps_sm")
            xnT3 = xnT[bb].rearrange("p (t l) -> p t l", t=2)
            for dt_ in range(2):
                nc.tensor.matmul(
                    out=psa, lhsT=w1sT[dt_],
                    rhs=xnT3[:, dt_, lt * 128:(lt + 1) * 128],
                    start=(dt_ == 0), stop=(dt_ == 1))
            nc.scalar.activation(out=aT[bb][:, lt * 128:(lt + 1) * 128],
                                 in_=psa, func=ACT_IDENT, bias=b1c)
            psA4 = ps_small.tile([128, O], F32, tag="ps_sm")
            nc.tensor.matmul(out=psA4,
                             lhsT=aT[bb][:, lt * 128:(lt + 1) * 128],
                             rhs=w4Tr, start=True, stop=True)
            nc.scalar.activation(out=A4[bb][lt], in_=psA4, func=ACT_COPY)

        for lt in range(4):
            for bb in range(B):
                xn = ln_normalize(x_t[(bb, lt)], 128, 0 if bb == 0 else 2)
                emit_x_pe_side(bb, lt, xn)

        # ---- V3 / V4m chunks: V4m on Pool, V3 on DVE, emitted per batch ----
        V3 = [big.tile([C, JS * O], F32R, name=f"V3_{b_}") for b_ in range(B)]
        V4m = [big.tile([C, JS * O], F32R, name=f"V4m{b_}") for b_ in range(B)]

        def emit_v(bb):
            for jb in range(8):
                sl = slice(jb * JB, (jb + 1) * JB)
                v3 = V3[bb].rearrange("c (j o) -> c j o", j=JS)[:, sl, :]
                v4 = V4m[bb].rearrange("c (j o) -> c j o", j=JS)[:, sl, :]
                bT3 = bT[bb][:, sl].unsqueeze(2).broadcast_to([C, JB, O])
                bTs3 = bTs[bb][:, sl].unsqueeze(2).broadcast_to([C, JB, O])
                w3b = w3T.unsqueeze(1).broadcast_to([C, JB, O])
                w4nb = w4Tn.unsqueeze(1).broadcast_to([C, JB, O])
                nc.gpsimd.tensor_tensor(out=v4, in0=bTs3, in1=w4nb,
                                        op=AluOpType.mult)
                nc.vector.tensor_tensor(out=v3, in0=bT3, in1=w3b,
                                        op=AluOpType.mult)

        # ---- main loop: per batch: V chunks then 4 i-tiles x 2 j-halves ----
        w4rb = w4Tr.unsqueeze(1).broadcast_to([C, JB, O])
        nepi = 0
        for bb in range(B):
            emit_v(bb)
            for it in range(4):
                lhs_a = aT[bb][:, it * 128:(it + 1) * 128]
                for half in range(2):
                    stage = stage_pool.tile([128, JS * O // 2], F32, tag="stage")
                    for jc in range(2):
                        on_act = nepi % 2 == 1
                        ps = ps_pool.tile([128, 1024], F32, tag="ps_main")
                        for h in range(2):
                            jb = half * 4 + jc * 2 + h
                            sec = ps[:, h * 512:(h + 1) * 512]
                            nc.tensor.matmul(
                                out=sec, lhsT=ones128,
                                rhs=V4m[bb][:, jb * 512:(jb + 1) * 512],
                                start=True, stop=False)
                            if on_act:
                                nc.tensor.matmul(out=sec, lhsT=lhs_a, rhs=w4rb,
                                                 start=False, stop=False)
                            nc.tensor.matmul(
                                out=sec, lhsT=lhs_a,
                                rhs=V3[bb][:, jb * 512:(jb + 1) * 512],
                                start=False, stop=True)
                        dst = stage[:, jc * 1024:(jc + 1) * 1024]
                        if on_act:
                            nc.scalar.activation(out=dst, in_=ps, func=ACT_COPY)
                        else:
                            a4b = A4[bb][it].unsqueeze(1).broadcast_to(
                                [128, 16, O])
                            ps3 = ps.rearrange("p (j o) -> p j o", j=16)
                            dst3 = dst.rearrange("p (j o) -> p j o", j=16)
                            nc.vector.tensor_tensor(out=dst3, in0=ps3, in1=a4b,
                                                    op=AluOpType.add)
                        nepi += 1
                    nc.sync.dma_start(
                        out=out.ap()[bb, it * 128:(it + 1) * 128,
                                     half * 32:(half + 1) * 32, :],
                        in_=stage.rearrange("p (j o) -> p j o", j=JS // 2))

    nc.compile()
    return nc


_NC = None


def _solve_b3_shift(w4, b3):
    """Exact min-norm t with w4 @ t = b3 (w4: [O, C], full row rank)."""
    w4d = np.asarray(w4, np.float64)
    b3d = np.asarray(b3, np.float64)
    try:
        t = w4d.T @ np.linalg.solve(w4d @ w4d.T, b3d)
    except np.linalg.LinAlgError:
        t = np.linalg.lstsq(w4d, b3d, rcond=None)[0]
    return t.astype(np.float32)


def kernel(**inputs):
    global _NC
    if _NC is None:
        _NC = build_nc()
    f32 = lambda v: np.asarray(v, dtype=np.float32)
    x = np.ascontiguousarray(inputs["x"], dtype=np.float32)
    norm_w, norm_b = f32(inputs["norm_w"]), f32(inputs["norm_b"])
    w1, w2 = f32(inputs["w1"]), f32(inputs["w2"])
    # fold the layernorm affine into the GEMM weights/biases (host, exact)
    w1s = np.ascontiguousarray(w1 * norm_w[None, :])
    w2s = np.ascontiguousarray(w2 * norm_w[None, :])
    b1f = f32(inputs["b1"]) + w1 @ norm_b
    b2f = f32(inputs["b2"]) + w2 @ norm_b
    t = _solve_b3_shift(inputs["w4"], inputs["b3"])
    common = {
        "x": x,
        "w12": np.ascontiguousarray(np.stack([w1s, w2s])),
        "w34": np.ascontiguousarray(
            np.stack([f32(inputs["w3"]), f32(inputs["w4"])])),
        "b123": np.ascontiguousarray(
            np.stack([b1f, b2f, b2f - t], axis=1)),
    }
    in_maps = []
    for k in range(NCORES):
        m = dict(common)
        m["xj"] = np.ascontiguousarray(x[k * JS:(k + 1) * JS], np.float32)
        in_maps.append(m)
    res = run_bass_kernel_spmd(_NC, in_maps, core_ids=list(range(NCORES)))
    return np.concatenate([res.results[k]["out"] for k in range(NCORES)], axis=2)
